# revision 35
# baseline (speedup 1.0000x reference)
"""Qwen-style GQA full attention (B=2, S=2048, HID=2048, H=16, KVH=8, D=128)
on 8 trn2 NeuronCores.

Sharding: tensor-parallel across head groups. Core d owns kv-head d and its
two query heads (2d, 2d+1): Wq/Wk/Wv column shards, Wo row shard. Each core
computes a partial [B*S, HID] output (its 2 heads' contribution through its
Wo row block); the host sums the 8 partials.

Device kernel (per core, all matmuls bf16, fp32 PSUM accumulation):
  phase 1  QKV+gate projection, feature-major ([feat, tok]) via stationary
           W-chunks against moving hsT (host-pretransposed hidden states).
           Per-head RMSNorm done with a ones-vector partition-sum matmul +
           exp(-0.5*ln(ss/128+eps)); RoPE via half-rotated sin/cos tables
           (norm weight + 1/sqrt(D) folded in host-side). Gate sigmoid is
           computed as exp(-ln(1+exp(-g))) so the scalar engine only ever
           needs the natural_log_exp table set.
  phase 2  V transposed to token-major via PE transposes.
  phase 3  causal attention per (batch, q-tile-pair): scoresT = K-chunk
           stationary x moving Q -> exp -> diagonal-block masking (exact
           zeros) -> PV and broadcast row-sum accumulation; out columns are
           rescaled by exp(-ln(sum)) (softmax denominator; no max
           subtraction needed since RMS-normed q,k bound |score|<=sqrt(D)).
  phase 4  sigmoid-gate multiply + Wo row-shard projection -> partial out.
"""

import os
import numpy as np
import ml_dtypes

import concourse.bass as bass
import concourse.tile as tile
from concourse import bacc, mybir
from contextlib import ExitStack

BF16 = ml_dtypes.bfloat16
F32 = mybir.dt.float32
BF = mybir.dt.bfloat16
AF = mybir.ActivationFunctionType

class _Bacc(bacc.Bacc):
    """Bacc that prefers the combined Ln+Exp activation table set, so the
    kernel's Ln/Exp/Copy mix resolves to a single ACT_TABLE_LOAD instead of
    thrashing between exp_and_others and natural_log (~2.7us per switch)."""

    def insert_act_table_loads(self):
        import bass_rust as _bass_rust
        from concourse.hw_specs import get_activation_tables
        has_activation = any(
            isinstance(i, mybir.InstActivation)
            for b in self.main_func.blocks
            for i in b.instructions
        )
        if not has_activation:
            return
        # act_func_set_id is positional: keep list order, but hide every
        # set except the combined one so the pass can only pick it.
        items = [
            (nm, fns if nm == "natural_log_exp_and_others" else set())
            for nm, fns in get_activation_tables(self.m.arch).items()
        ]
        _bass_rust.insert_act_table_loads(self, items)


B, S, HID, H, KVH, D = 2, 2048, 2048, 16, 8, 128
G = H // KVH              # q heads per kv head (= per core)
EPS = 1e-6
SCALE = D ** -0.5
CH = 512                  # token chunk (proj phase)
NCORES = 8


def build_nc(S_=S):
    """Build the single-core SPMD program (identical on all 8 cores)."""
    HC = HID // 128           # hid chunks
    N = B * S_                # total tokens
    SK = S_ // 128            # k-tiles per batch
    NP = S_ // 256            # q-tile pairs per batch
    CPB = S_ // CH            # token chunks per batch
    NT = CH // 128            # 128-tok tiles per chunk

    nc = _Bacc(None)
    nc._phase_marks = []
    _mark = lambda s: nc._phase_marks.append((s, nc.next_id()))

    hsT_d = nc.dram_tensor("hsT", [HID, N], BF, kind="ExternalInput")
    wq_d = nc.dram_tensor("wq", [HC, 128, 512], BF, kind="ExternalInput")
    wk_d = nc.dram_tensor("wk", [HC, 128, 128], BF, kind="ExternalInput")
    wv_d = nc.dram_tensor("wv", [HC, 128, 128], BF, kind="ExternalInput")
    wo_d = nc.dram_tensor("wo", [G, 128, HID], BF, kind="ExternalInput")
    cq_d = nc.dram_tensor("cosq", [128, S_], BF, kind="ExternalInput")
    sq_d = nc.dram_tensor("sinq", [128, S_], BF, kind="ExternalInput")
    ck_d = nc.dram_tensor("cosk", [128, S_], BF, kind="ExternalInput")
    sk_d = nc.dram_tensor("sink", [128, S_], BF, kind="ExternalInput")
    id_d = nc.dram_tensor("ident", [128, 128], BF, kind="ExternalInput")
    o1_d = nc.dram_tensor("ones1", [128, 1], BF, kind="ExternalInput")
    ob_d = nc.dram_tensor("onesb", [1, 128], BF, kind="ExternalInput")
    o128_d = nc.dram_tensor("ones128", [128, 128], BF, kind="ExternalInput")
    ma_d = nc.dram_tensor("maska", [128, 512], BF, kind="ExternalInput")
    mb_d = nc.dram_tensor("maskb", [128, 512], BF, kind="ExternalInput")
    out_d = nc.dram_tensor("out", [N, HID], BF, kind="ExternalOutput")

    with tile.TileContext(nc) as tc, ExitStack() as ctx:
        cpool = ctx.enter_context(tc.tile_pool(name="consts", bufs=1))

        wq_s = cpool.tile([128, HC, 512], BF)
        wk_s = cpool.tile([128, HC, 128], BF)
        wv_s = cpool.tile([128, HC, 128], BF)
        wo_s = cpool.tile([128, G, HID], BF)
        cq_s = cpool.tile([128, S_], BF)
        sq_s = cpool.tile([128, S_], BF)
        ck_s = cpool.tile([128, S_], BF)
        sk_s = cpool.tile([128, S_], BF)
        id_s = cpool.tile([128, 128], BF)
        o1_s = cpool.tile([128, 1], BF)
        ob_s = cpool.tile([1, 128], BF)
        o128_s = cpool.tile([128, 128], BF)
        ma_s = cpool.tile([128, 512], BF)
        mb_s = cpool.tile([128, 512], BF)
        epsb = cpool.tile([128, 1], F32)
        oneb = cpool.tile([128, 1], F32)
        nc.vector.memset(epsb[:], EPS)
        nc.vector.memset(oneb[:], 1.0)

        # per-chunk weight loads so the first projection matmuls unblock
        # as soon as their own W chunk lands (not the whole 2MB tensor)
        for c in range(HC):
            nc.sync.dma_start(wq_s[:, c, :], wq_d[c])
            nc.scalar.dma_start(wk_s[:, c, :], wk_d[c])
            nc.scalar.dma_start(wv_s[:, c, :], wv_d[c])
        nc.sync.dma_start(wo_s[:], wo_d[:].rearrange("c p f -> p c f"))
        for dst, src in ((cq_s, cq_d), (sq_s, sq_d), (ck_s, ck_d), (sk_s, sk_d),
                         (id_s, id_d), (o1_s, o1_d), (ob_s, ob_d),
                         (o128_s, o128_d), (ma_s, ma_d), (mb_s, mb_d)):
            nc.sync.dma_start(dst[:], src[:])

        _mark('consts')
        # persistent activations (feature-major: [D, ...tok])
        qtb = cpool.tile([128, B, SK, G, 128], BF)   # rope'd+normed q
        ktb = cpool.tile([128, B, SK, 128], BF)      # rope'd+normed k
        vtb = cpool.tile([128, N], BF)               # v, feature-major
        vb = cpool.tile([128, B, SK, 128], BF)       # v, token-major
        gtb = cpool.tile([128, B, SK, G, 128], BF)   # sigmoid(gate)

        # ---------------- phase 1: projections ----------------
        hsT_v = hsT_d[:].rearrange("(c p) n -> c p n", p=128)
        with (
            tc.tile_pool(name="hst", bufs=2) as hstp,
            tc.tile_pool(name="projps", bufs=6, space="PSUM") as projps,
            tc.tile_pool(name="ssps", bufs=1, space="PSUM") as ssps,
            tc.tile_pool(name="auxps", bufs=1, space="PSUM") as auxps,
            tc.tile_pool(name="pwork", bufs=3) as pwork,
        ):
            for b in range(B):
                for cc in range(CPB):
                    t0 = b * S_ + cc * CH     # global token start
                    p0 = cc * CH              # position start (within batch)
                    ht = hstp.tile([128, HC, CH], BF, tag="hst")
                    for c4 in range(0, HC, 4):
                        nc.gpsimd.dma_start(
                            ht[:, c4:c4 + 4, :],
                            hsT_v[c4:c4 + 4, :, t0:t0 + CH].rearrange(
                                "c p f -> p c f"))
                    hts = [ht[:, c, :] for c in range(HC)]

                    psq0 = projps.tile([128, CH], F32, tag="pp")
                    psq1 = projps.tile([128, CH], F32, tag="pp")
                    psk = projps.tile([128, CH], F32, tag="pp")
                    psv = projps.tile([128, CH], F32, tag="pp")
                    psg0 = projps.tile([128, CH], F32, tag="pp")
                    psg1 = projps.tile([128, CH], F32, tag="pp")
                    for c in range(HC):
                        st, sp = c == 0, c == HC - 1
                        nc.tensor.matmul(psq0[:], wq_s[:, c, 0:128], hts[c],
                                         start=st, stop=sp)
                        nc.tensor.matmul(psq1[:], wq_s[:, c, 128:256], hts[c],
                                         start=st, stop=sp)
                        nc.tensor.matmul(psk[:], wk_s[:, c, :], hts[c],
                                         start=st, stop=sp)
                        nc.tensor.matmul(psv[:], wv_s[:, c, :], hts[c],
                                         start=st, stop=sp)
                        nc.tensor.matmul(psg0[:], wq_s[:, c, 256:384], hts[c],
                                         start=st, stop=sp)
                        nc.tensor.matmul(psg1[:], wq_s[:, c, 384:512], hts[c],
                                         start=st, stop=sp)

                    ti0 = cc * NT
                    # RMSNorm + RoPE for q heads and k
                    blocks = [
                        (psq0, cq_s, sq_s, qtb[:, b, ti0:ti0 + NT, 0, :]),
                        (psq1, cq_s, sq_s, qtb[:, b, ti0:ti0 + NT, 1, :]),
                        (psk, ck_s, sk_s, ktb[:, b, ti0:ti0 + NT, :]),
                    ]
                    for psx, ctab, stab, dest in blocks:
                        xu = pwork.tile([128, CH], BF, tag="xu")
                        nc.scalar.copy(xu[:], psx[:])
                        xsq = pwork.tile([128, CH], BF, tag="xsq")
                        nc.vector.tensor_mul(xsq[:], xu[:], xu[:])
                        ssp = ssps.tile([1, CH], F32, tag="ss")
                        nc.tensor.matmul(ssp[:], o1_s[:], xsq[:])
                        ssl = pwork.tile([1, CH], F32, tag="ssl")
                        nc.scalar.activation(ssl[:], ssp[:], AF.Ln,
                                             bias=epsb[:1], scale=1.0 / D)
                        rsts = pwork.tile([1, CH], BF, tag="rsts")
                        nc.scalar.activation(rsts[:], ssl[:], AF.Exp, scale=-0.5)
                        rstdB = auxps.tile([128, CH], F32, tag="aux")
                        nc.tensor.matmul(rstdB[:], ob_s[:], rsts[:])
                        t1 = pwork.tile([128, CH], BF, tag="t1")
                        nc.vector.tensor_mul(t1[:], xu[:], ctab[:, p0:p0 + CH])
                        xrot = pwork.tile([128, CH], BF, tag="xrot")
                        nc.vector.tensor_copy(xrot[0:64, :], xu[64:128, :])
                        nc.vector.tensor_copy(xrot[64:128, :], xu[0:64, :])
                        t2 = pwork.tile([128, CH], BF, tag="t2")
                        nc.vector.tensor_mul(t2[:], xrot[:],
                                             stab[:, p0:p0 + CH])
                        nc.vector.tensor_add(t1[:], t1[:], t2[:])
                        nc.vector.tensor_mul(dest, t1[:], rstdB[:])

                    # v: stash feature-major (transposed later)
                    nc.scalar.copy(vtb[:, t0:t0 + CH], psv[:])

                    # gates: sigmoid(g) = exp(-ln(1 + exp(-g)))
                    for hh, psg in ((0, psg0), (1, psg1)):
                        e1 = pwork.tile([128, CH], BF, tag="e1")
                        nc.scalar.activation(e1[:], psg[:], AF.Exp, scale=-1.0)
                        l1 = pwork.tile([128, CH], F32, tag="l1")
                        nc.scalar.activation(l1[:], e1[:], AF.Ln, bias=oneb[:])
                        nc.scalar.activation(gtb[:, b, ti0:ti0 + NT, hh, :],
                                             l1[:], AF.Exp, scale=-1.0)
                    _mark(f'proj b{b}c{cc}')

            # ---------------- phase 2: V -> token-major ----------------
            for b in range(B):
                for j4 in range(0, SK, 4):
                    vt_ps = auxps.tile([128, 512], BF, tag="aux",
                                       name="vt_ps")
                    for jj in range(4):
                        j = j4 + jj
                        nc.tensor.transpose(
                            vt_ps[:, jj * 128:(jj + 1) * 128],
                            vtb[:, b * S_ + j * 128: b * S_ + (j + 1) * 128],
                            id_s[:])
                    nc.scalar.copy(vb[:, b, j4:j4 + 4, :], vt_ps[:])

        _mark('vtrans')
        # ---------------- phase 3+4: attention + gating + Wo ----------------
        with (
            tc.tile_pool(name="scps", bufs=2, space="PSUM") as scps,
            tc.tile_pool(name="pvps", bufs=2, space="PSUM") as pvps,
            tc.tile_pool(name="sumps", bufs=2, space="PSUM") as sumps,
            tc.tile_pool(name="wops", bufs=2, space="PSUM") as wops,
            tc.tile_pool(name="probsp", bufs=6) as probsp,
            tc.tile_pool(name="awork", bufs=3) as awork,
        ):
            def wo_proj(b, i0, gated):
                # gating result of pair (b, i0//2) -> Wo row-shard -> DRAM
                for it in range(2):
                    trow = b * S_ + (i0 + it) * 128
                    osb = awork.tile([128, HID], BF, tag="osb")
                    for oc in range(HID // 512):
                        wop = wops.tile([128, 512], F32, tag="wo")
                        nc.tensor.matmul(
                            wop[:], gated[:, it * 256:it * 256 + 128],
                            wo_s[:, 0, oc * 512:(oc + 1) * 512],
                            start=True, stop=False)
                        nc.tensor.matmul(
                            wop[:], gated[:, it * 256 + 128:it * 256 + 256],
                            wo_s[:, 1, oc * 512:(oc + 1) * 512],
                            start=False, stop=True)
                        nc.vector.tensor_copy(
                            osb[:, oc * 512:(oc + 1) * 512], wop[:])
                    nc.gpsimd.dma_start(out_d[trow:trow + 128, :], osb[:])

            # Two-pair interleaved attention: pairs (2g, 2g+1) advance their
            # j-loops together (independent psum accumulators), so the PE has
            # ~6 matmuls in flight per j step to hide each exp's latency.
            # The Wo projection of the previous group is emitted after the
            # current group's attention as additional filler.
            def attn_pair(st, j):
                b, i0, jmax, pv, smp = st
                scp = scps.tile([128, 512], F32, tag="sc")
                nc.tensor.matmul(scp[:], ktb[:, b, j, :],
                                 qtb[:, b, i0:i0 + 2, :, :])
                probs = probsp.tile([128, 512], BF, tag="probs")
                nc.scalar.activation(probs[:], scp[:], AF.Exp)
                if j == i0:
                    nc.vector.tensor_mul(probs[:], probs[:], ma_s[:])
                elif j == jmax:
                    nc.vector.tensor_mul(probs[:], probs[:], mb_s[:])
                nc.tensor.matmul(pv[:], vb[:, b, j, :], probs[:],
                                 start=(j == 0), stop=(j == jmax))
                nc.tensor.matmul(smp[:], o128_s[:], probs[:],
                                 start=(j == 0), stop=(j == jmax))

            def gate_pair(st):
                b, i0, jmax, pv, smp = st
                lsb = awork.tile([128, 512], F32, tag="lsb")
                nc.scalar.activation(lsb[:], smp[:], AF.Ln)
                rsb = awork.tile([128, 512], F32, tag="rsb")
                nc.scalar.activation(rsb[:], lsb[:], AF.Exp, scale=-1.0)
                tmp = awork.tile([128, 512], BF, tag="tmp")
                nc.vector.tensor_mul(tmp[:], pv[:], rsb[:])
                gated = probsp.tile([128, 512], BF, tag="gated")
                nc.vector.tensor_mul(gated[:], tmp[:],
                                     gtb[:, b, i0:i0 + 2, :, :])
                return (b, i0, gated)

            pending = []
            for b in range(B):
                for pA in range(0, NP, 2):
                    pB = pA + 1
                    sts = []
                    for p in (pA, pB):
                        i0 = 2 * p
                        sts.append((b, i0, i0 + 1,
                                    pvps.tile([128, 512], F32, tag="pv",
                                              name="pv"),
                                    sumps.tile([128, 512], F32, tag="sm",
                                               name="sm")))
                    stA, stB = sts
                    done = []
                    for j in range(stB[2] + 1):
                        if j <= stA[2]:
                            attn_pair(stA, j)
                        attn_pair(stB, j)
                        if j == stA[2]:
                            done.append(gate_pair(stA))
                            if pending:
                                wo_proj(*pending.pop(0))
                    done.append(gate_pair(stB))
                    _mark(f'attn b{b}g{pA//2}')
                    for pend in pending:
                        wo_proj(*pend)
                    pending = done
            for pend in pending:
                wo_proj(*pend)
    nc.compile()
    return nc


def prep_inputs(hidden_states, cos, sin, Wq, Wk, Wv, Wo, q_norm_w, k_norm_w,
                S_=S):
    """Host-side sharding + layout prep. Returns in_maps for 8 cores."""
    N = B * S_
    hsT = np.ascontiguousarray(
        hidden_states.reshape(N, HID).T).astype(BF16)

    cos0 = np.asarray(cos[0], np.float32)   # [S_, D] (identical across batch)
    sin0 = np.asarray(sin[0], np.float32)
    qw = np.asarray(q_norm_w, np.float32)
    kw = np.asarray(k_norm_w, np.float32)
    sign = np.where(np.arange(D) < 64, -1.0, 1.0).astype(np.float32)
    shift = (np.arange(D) + 64) % D

    cosq = np.ascontiguousarray(cos0.T * qw[:, None] * SCALE).astype(BF16)
    sinq = np.ascontiguousarray(
        sin0.T * (sign * qw[shift])[:, None] * SCALE).astype(BF16)
    cosk = np.ascontiguousarray(cos0.T * kw[:, None]).astype(BF16)
    sink = np.ascontiguousarray(
        sin0.T * (sign * kw[shift])[:, None]).astype(BF16)

    tri = (np.arange(128)[:, None] <= np.arange(128)[None, :])
    onesq = np.ones((128, 128), np.float32)
    maska = np.concatenate([tri, tri, onesq, onesq], axis=1).astype(BF16)
    maskb = np.concatenate([0 * onesq, 0 * onesq, tri, tri],
                           axis=1).astype(BF16)
    ident = np.eye(128, dtype=BF16)
    ones1 = np.ones((128, 1), BF16)
    onesb = np.ones((1, 128), BF16)
    ones128 = np.ones((128, 128), BF16)

    HC = HID // 128
    in_maps = []
    for d in range(NCORES):
        h0, h1 = G * d, G * d + 1
        q0 = Wq[:, h0 * 2 * D: h0 * 2 * D + D]
        g0 = Wq[:, h0 * 2 * D + D: (h0 + 1) * 2 * D]
        q1 = Wq[:, h1 * 2 * D: h1 * 2 * D + D]
        g1 = Wq[:, h1 * 2 * D + D: (h1 + 1) * 2 * D]
        wq_c = np.concatenate([q0, q1, g0, g1], axis=1)      # [HID, 512]
        wq_a = np.ascontiguousarray(wq_c).astype(BF16).reshape(HC, 128, 512)
        wk_a = np.ascontiguousarray(
            Wk[:, d * D:(d + 1) * D]).astype(BF16).reshape(HC, 128, 128)
        wv_a = np.ascontiguousarray(
            Wv[:, d * D:(d + 1) * D]).astype(BF16).reshape(HC, 128, 128)
        wo_a = np.ascontiguousarray(
            Wo[d * G * D:(d + 1) * G * D, :]).astype(BF16).reshape(G, 128, HID)
        in_maps.append({
            "hsT": hsT, "wq": wq_a, "wk": wk_a, "wv": wv_a, "wo": wo_a,
            "cosq": cosq, "sinq": sinq, "cosk": cosk, "sink": sink,
            "ident": ident, "ones1": ones1, "onesb": onesb,
            "ones128": ones128, "maska": maska, "maskb": maskb,
        })
    return in_maps


_NC_CACHE = {}
_RUNNER_CACHE = {}


def _get_nc(S_=S):
    if S_ not in _NC_CACHE:
        _NC_CACHE[S_] = build_nc(S_)
    return _NC_CACHE[S_]


def _get_runner(S_=S):
    """Build a cached jitted 8-core executable.

    Mirrors concourse.bass2jax.run_bass_via_pjrt's multi-core path, but
    keeps the jitted function (and device-resident output placeholders)
    so repeated calls don't re-trace/re-compile, and so the executable
    can be timed in a steady-state loop.
    """
    if S_ in _RUNNER_CACHE:
        return _RUNNER_CACHE[S_]
    import jax
    from jax.experimental.shard_map import shard_map
    from jax.sharding import Mesh, PartitionSpec
    from concourse import bass2jax, mybir as _mybir
    bass2jax.install_neuronx_cc_hook()

    nc = _get_nc(S_)
    assert nc.dbg_addr is None
    pid_name = (nc.partition_id_tensor.name
                if nc.partition_id_tensor is not None else None)

    in_names, out_names, out_avals = [], [], []
    for alloc in nc.m.functions[0].allocations:
        if not isinstance(alloc, _mybir.MemoryLocationSet):
            continue
        name = alloc.memorylocations[0].name
        if alloc.kind == "ExternalInput":
            if name != pid_name:
                in_names.append(name)
        elif alloc.kind == "ExternalOutput":
            out_names.append(name)
            out_avals.append(jax.core.ShapedArray(
                tuple(alloc.tensor_shape), _mybir.dt.np(alloc.dtype)))
    n_params = len(in_names)
    all_names = in_names + out_names
    if pid_name is not None:
        all_names = all_names + [pid_name]

    def _body(*args):
        operands = list(args)
        if pid_name is not None:
            operands.append(bass2jax.partition_id_tensor())
        outs = bass2jax._bass_exec_p.bind(
            *operands,
            out_avals=tuple(out_avals),
            in_names=tuple(all_names),
            out_names=tuple(out_names),
            lowering_input_output_aliases=(),
            sim_require_finite=True,
            sim_require_nnan=True,
            nc=nc,
        )
        return tuple(outs)

    devices = jax.devices()[:NCORES]
    mesh = Mesh(np.asarray(devices), ("core",))
    nin = n_params + len(out_names)
    sharded = jax.jit(
        shard_map(_body, mesh=mesh,
                  in_specs=(PartitionSpec("core"),) * nin,
                  out_specs=(PartitionSpec("core"),) * len(out_names),
                  check_rep=False),
        keep_unused=True,
    )
    zeros = [np.zeros((NCORES * a.shape[0], *a.shape[1:]), a.dtype)
             for a in out_avals]
    zeros_dev = [jax.device_put(z) for z in zeros]

    def run(in_maps):
        concat_in = [
            np.concatenate([np.asarray(m[nm]) for m in in_maps], axis=0)
            for nm in in_names
        ]
        outs = sharded(*concat_in, *zeros_dev)
        return {nm: np.asarray(outs[i]) for i, nm in enumerate(out_names)}

    def run_prepared(dev_args):
        return sharded(*dev_args, *zeros_dev)

    def prepare(in_maps):
        return [
            jax.device_put(np.concatenate(
                [np.asarray(m[nm]) for m in in_maps], axis=0))
            for nm in in_names
        ]

    r = {"run": run, "prepare": prepare, "run_prepared": run_prepared,
         "out_names": out_names, "out_avals": out_avals}
    _RUNNER_CACHE[S_] = r
    return r


def kernel(hidden_states, cos, sin, Wq, Wk, Wv, Wo, q_norm_w, k_norm_w):
    in_maps = prep_inputs(hidden_states, cos, sin, Wq, Wk, Wv, Wo,
                          q_norm_w, k_norm_w)
    runner = _get_runner()
    outs = runner["run"](in_maps)
    full = outs["out"].reshape(NCORES, B * S, HID)
    acc = full.astype(np.float32).sum(axis=0)
    return acc.reshape(B, S, HID)


# revision 36
# speedup vs baseline: 1.0075x; 1.0075x over previous
"""Qwen-style GQA full attention (B=2, S=2048, HID=2048, H=16, KVH=8, D=128)
on 8 trn2 NeuronCores.

Sharding: tensor-parallel across head groups. Core d owns kv-head d and its
two query heads (2d, 2d+1): Wq/Wk/Wv column shards, Wo row shard. Each core
computes a partial [B*S, HID] output (its 2 heads' contribution through its
Wo row block); the host sums the 8 partials.

Device kernel (per core, all matmuls bf16, fp32 PSUM accumulation):
  phase 1  QKV+gate projection, feature-major ([feat, tok]) via stationary
           W-chunks against moving hsT (host-pretransposed hidden states).
           Per-head RMSNorm done with a ones-vector partition-sum matmul +
           exp(-0.5*ln(ss/128+eps)); RoPE via half-rotated sin/cos tables
           (norm weight + 1/sqrt(D) folded in host-side). Gate sigmoid is
           computed as exp(-ln(1+exp(-g))) so the scalar engine only ever
           needs the natural_log_exp table set.
  phase 2  V transposed to token-major via PE transposes.
  phase 3  causal attention per (batch, q-tile-pair): scoresT = K-chunk
           stationary x moving Q -> exp -> diagonal-block masking (exact
           zeros) -> PV and broadcast row-sum accumulation; out columns are
           rescaled by exp(-ln(sum)) (softmax denominator; no max
           subtraction needed since RMS-normed q,k bound |score|<=sqrt(D)).
  phase 4  sigmoid-gate multiply + Wo row-shard projection -> partial out.
"""

import os
import numpy as np
import ml_dtypes

import concourse.bass as bass
import concourse.tile as tile
from concourse import bacc, mybir
from contextlib import ExitStack

BF16 = ml_dtypes.bfloat16
F32 = mybir.dt.float32
BF = mybir.dt.bfloat16
AF = mybir.ActivationFunctionType

class _Bacc(bacc.Bacc):
    """Bacc that prefers the combined Ln+Exp activation table set, so the
    kernel's Ln/Exp/Copy mix resolves to a single ACT_TABLE_LOAD instead of
    thrashing between exp_and_others and natural_log (~2.7us per switch)."""

    def insert_act_table_loads(self):
        import bass_rust as _bass_rust
        from concourse.hw_specs import get_activation_tables
        has_activation = any(
            isinstance(i, mybir.InstActivation)
            for b in self.main_func.blocks
            for i in b.instructions
        )
        if not has_activation:
            return
        # act_func_set_id is positional: keep list order, but hide every
        # set except the combined one so the pass can only pick it.
        items = [
            (nm, fns if nm == "natural_log_exp_and_others" else set())
            for nm, fns in get_activation_tables(self.m.arch).items()
        ]
        _bass_rust.insert_act_table_loads(self, items)


B, S, HID, H, KVH, D = 2, 2048, 2048, 16, 8, 128
G = H // KVH              # q heads per kv head (= per core)
EPS = 1e-6
SCALE = D ** -0.5
CH = 512                  # token chunk (proj phase)
NCORES = 8


def build_nc(S_=S):
    """Build the single-core SPMD program (identical on all 8 cores)."""
    HC = HID // 128           # hid chunks
    N = B * S_                # total tokens
    SK = S_ // 128            # k-tiles per batch
    NP = S_ // 256            # q-tile pairs per batch
    CPB = S_ // CH            # token chunks per batch
    NT = CH // 128            # 128-tok tiles per chunk

    nc = _Bacc(None)
    nc._phase_marks = []
    _mark = lambda s: nc._phase_marks.append((s, nc.next_id()))

    hsT_d = nc.dram_tensor("hsT", [HID, N], BF, kind="ExternalInput")
    wq_d = nc.dram_tensor("wq", [HC, 128, 512], BF, kind="ExternalInput")
    wk_d = nc.dram_tensor("wk", [HC, 128, 128], BF, kind="ExternalInput")
    wv_d = nc.dram_tensor("wv", [HC, 128, 128], BF, kind="ExternalInput")
    wo_d = nc.dram_tensor("wo", [G, 128, HID], BF, kind="ExternalInput")
    cq_d = nc.dram_tensor("cosq", [128, S_], BF, kind="ExternalInput")
    sq_d = nc.dram_tensor("sinq", [128, S_], BF, kind="ExternalInput")
    ck_d = nc.dram_tensor("cosk", [128, S_], BF, kind="ExternalInput")
    sk_d = nc.dram_tensor("sink", [128, S_], BF, kind="ExternalInput")
    id_d = nc.dram_tensor("ident", [128, 128], BF, kind="ExternalInput")
    o1_d = nc.dram_tensor("ones1", [128, 1], BF, kind="ExternalInput")
    ob_d = nc.dram_tensor("onesb", [1, 128], BF, kind="ExternalInput")
    o128_d = nc.dram_tensor("ones128", [128, 128], BF, kind="ExternalInput")
    ma_d = nc.dram_tensor("maska", [128, 512], BF, kind="ExternalInput")
    mb_d = nc.dram_tensor("maskb", [128, 512], BF, kind="ExternalInput")
    out_d = nc.dram_tensor("out", [N, HID], BF, kind="ExternalOutput")

    with tile.TileContext(nc) as tc, ExitStack() as ctx:
        cpool = ctx.enter_context(tc.tile_pool(name="consts", bufs=1))

        wq_s = cpool.tile([128, HC, 512], BF)
        wk_s = cpool.tile([128, HC, 128], BF)
        wv_s = cpool.tile([128, HC, 128], BF)
        wo_s = cpool.tile([128, G, HID], BF)
        cq_s = cpool.tile([128, S_], BF)
        sq_s = cpool.tile([128, S_], BF)
        ck_s = cpool.tile([128, S_], BF)
        sk_s = cpool.tile([128, S_], BF)
        id_s = cpool.tile([128, 128], BF)
        o1_s = cpool.tile([128, 1], BF)
        ob_s = cpool.tile([1, 128], BF)
        o128_s = cpool.tile([128, 128], BF)
        ma_s = cpool.tile([128, 512], BF)
        mb_s = cpool.tile([128, 512], BF)
        epsb = cpool.tile([128, 1], F32)
        oneb = cpool.tile([128, 1], F32)
        nc.vector.memset(epsb[:], EPS)
        nc.vector.memset(oneb[:], 1.0)

        # per-chunk weight loads so the first projection matmuls unblock
        # as soon as their own W chunk lands (not the whole 2MB tensor)
        for c in range(HC):
            nc.sync.dma_start(wq_s[:, c, :], wq_d[c])
            nc.scalar.dma_start(wk_s[:, c, :], wk_d[c])
            nc.scalar.dma_start(wv_s[:, c, :], wv_d[c])
        nc.sync.dma_start(wo_s[:], wo_d[:].rearrange("c p f -> p c f"))
        for dst, src in ((cq_s, cq_d), (sq_s, sq_d), (ck_s, ck_d), (sk_s, sk_d),
                         (id_s, id_d), (o1_s, o1_d), (ob_s, ob_d),
                         (o128_s, o128_d), (ma_s, ma_d), (mb_s, mb_d)):
            nc.sync.dma_start(dst[:], src[:])

        _mark('consts')
        # persistent activations (feature-major: [D, ...tok])
        qtb = cpool.tile([128, B, SK, G, 128], BF)   # rope'd+normed q
        ktb = cpool.tile([128, B, SK, 128], BF)      # rope'd+normed k
        vtb = cpool.tile([128, N], BF)               # v, feature-major
        vb = cpool.tile([128, B, SK, 128], BF)       # v, token-major
        gtb = cpool.tile([128, B, SK, G, 128], BF)   # sigmoid(gate)

        # ---------------- phase 1: projections ----------------
        hsT_v = hsT_d[:].rearrange("(c p) n -> c p n", p=128)
        with (
            tc.tile_pool(name="hst", bufs=2) as hstp,
            tc.tile_pool(name="projps", bufs=6, space="PSUM") as projps,
            tc.tile_pool(name="ssps", bufs=1, space="PSUM") as ssps,
            tc.tile_pool(name="auxps", bufs=1, space="PSUM") as auxps,
            tc.tile_pool(name="pwork", bufs=3) as pwork,
        ):
            for b in range(B):
                for cc in range(CPB):
                    t0 = b * S_ + cc * CH     # global token start
                    p0 = cc * CH              # position start (within batch)
                    ht = hstp.tile([128, HC, CH], BF, tag="hst")
                    # finer sub-DMAs for the very first chunk so the first
                    # projection matmuls unblock as early as possible
                    step = 2 if (b == 0 and cc == 0) else 4
                    for c4 in range(0, HC, step):
                        nc.gpsimd.dma_start(
                            ht[:, c4:c4 + step, :],
                            hsT_v[c4:c4 + step, :, t0:t0 + CH].rearrange(
                                "c p f -> p c f"))
                    hts = [ht[:, c, :] for c in range(HC)]

                    psq0 = projps.tile([128, CH], F32, tag="pp")
                    psq1 = projps.tile([128, CH], F32, tag="pp")
                    psk = projps.tile([128, CH], F32, tag="pp")
                    psv = projps.tile([128, CH], F32, tag="pp")
                    psg0 = projps.tile([128, CH], F32, tag="pp")
                    psg1 = projps.tile([128, CH], F32, tag="pp")
                    for c in range(HC):
                        st, sp = c == 0, c == HC - 1
                        nc.tensor.matmul(psq0[:], wq_s[:, c, 0:128], hts[c],
                                         start=st, stop=sp)
                        nc.tensor.matmul(psq1[:], wq_s[:, c, 128:256], hts[c],
                                         start=st, stop=sp)
                        nc.tensor.matmul(psk[:], wk_s[:, c, :], hts[c],
                                         start=st, stop=sp)
                        nc.tensor.matmul(psv[:], wv_s[:, c, :], hts[c],
                                         start=st, stop=sp)
                        nc.tensor.matmul(psg0[:], wq_s[:, c, 256:384], hts[c],
                                         start=st, stop=sp)
                        nc.tensor.matmul(psg1[:], wq_s[:, c, 384:512], hts[c],
                                         start=st, stop=sp)

                    ti0 = cc * NT
                    # RMSNorm + RoPE for q heads and k
                    blocks = [
                        (psq0, cq_s, sq_s, qtb[:, b, ti0:ti0 + NT, 0, :]),
                        (psq1, cq_s, sq_s, qtb[:, b, ti0:ti0 + NT, 1, :]),
                        (psk, ck_s, sk_s, ktb[:, b, ti0:ti0 + NT, :]),
                    ]
                    for psx, ctab, stab, dest in blocks:
                        xu = pwork.tile([128, CH], BF, tag="xu")
                        nc.scalar.copy(xu[:], psx[:])
                        xsq = pwork.tile([128, CH], BF, tag="xsq")
                        nc.vector.tensor_mul(xsq[:], xu[:], xu[:])
                        ssp = ssps.tile([1, CH], F32, tag="ss")
                        nc.tensor.matmul(ssp[:], o1_s[:], xsq[:])
                        ssl = pwork.tile([1, CH], F32, tag="ssl")
                        nc.scalar.activation(ssl[:], ssp[:], AF.Ln,
                                             bias=epsb[:1], scale=1.0 / D)
                        rsts = pwork.tile([1, CH], BF, tag="rsts")
                        nc.scalar.activation(rsts[:], ssl[:], AF.Exp, scale=-0.5)
                        rstdB = auxps.tile([128, CH], F32, tag="aux")
                        nc.tensor.matmul(rstdB[:], ob_s[:], rsts[:])
                        t1 = pwork.tile([128, CH], BF, tag="t1")
                        nc.vector.tensor_mul(t1[:], xu[:], ctab[:, p0:p0 + CH])
                        xrot = pwork.tile([128, CH], BF, tag="xrot")
                        nc.vector.tensor_copy(xrot[0:64, :], xu[64:128, :])
                        nc.vector.tensor_copy(xrot[64:128, :], xu[0:64, :])
                        t2 = pwork.tile([128, CH], BF, tag="t2")
                        nc.vector.tensor_mul(t2[:], xrot[:],
                                             stab[:, p0:p0 + CH])
                        nc.vector.tensor_add(t1[:], t1[:], t2[:])
                        nc.vector.tensor_mul(dest, t1[:], rstdB[:])

                    # v: stash feature-major (transposed later)
                    nc.scalar.copy(vtb[:, t0:t0 + CH], psv[:])

                    # gates: sigmoid(g) = exp(-ln(1 + exp(-g)))
                    for hh, psg in ((0, psg0), (1, psg1)):
                        e1 = pwork.tile([128, CH], BF, tag="e1")
                        nc.scalar.activation(e1[:], psg[:], AF.Exp, scale=-1.0)
                        l1 = pwork.tile([128, CH], F32, tag="l1")
                        nc.scalar.activation(l1[:], e1[:], AF.Ln, bias=oneb[:])
                        nc.scalar.activation(gtb[:, b, ti0:ti0 + NT, hh, :],
                                             l1[:], AF.Exp, scale=-1.0)
                    _mark(f'proj b{b}c{cc}')

            # ---------------- phase 2: V -> token-major ----------------
            for b in range(B):
                for j4 in range(0, SK, 4):
                    vt_ps = auxps.tile([128, 512], BF, tag="aux",
                                       name="vt_ps")
                    for jj in range(4):
                        j = j4 + jj
                        nc.tensor.transpose(
                            vt_ps[:, jj * 128:(jj + 1) * 128],
                            vtb[:, b * S_ + j * 128: b * S_ + (j + 1) * 128],
                            id_s[:])
                    nc.scalar.copy(vb[:, b, j4:j4 + 4, :], vt_ps[:])

        _mark('vtrans')
        # ---------------- phase 3+4: attention + gating + Wo ----------------
        with (
            tc.tile_pool(name="scps", bufs=2, space="PSUM") as scps,
            tc.tile_pool(name="pvps", bufs=2, space="PSUM") as pvps,
            tc.tile_pool(name="sumps", bufs=2, space="PSUM") as sumps,
            tc.tile_pool(name="wops", bufs=2, space="PSUM") as wops,
            tc.tile_pool(name="probsp", bufs=6) as probsp,
            tc.tile_pool(name="awork", bufs=3) as awork,
        ):
            def wo_proj(b, i0, gated):
                # gating result of pair (b, i0//2) -> Wo row-shard -> DRAM
                for it in range(2):
                    trow = b * S_ + (i0 + it) * 128
                    osb = awork.tile([128, HID], BF, tag="osb")
                    for oc in range(HID // 512):
                        wop = wops.tile([128, 512], F32, tag="wo")
                        nc.tensor.matmul(
                            wop[:], gated[:, it * 256:it * 256 + 128],
                            wo_s[:, 0, oc * 512:(oc + 1) * 512],
                            start=True, stop=False)
                        nc.tensor.matmul(
                            wop[:], gated[:, it * 256 + 128:it * 256 + 256],
                            wo_s[:, 1, oc * 512:(oc + 1) * 512],
                            start=False, stop=True)
                        nc.vector.tensor_copy(
                            osb[:, oc * 512:(oc + 1) * 512], wop[:])
                    nc.gpsimd.dma_start(out_d[trow:trow + 128, :], osb[:])

            # Two-pair interleaved attention: pairs (2g, 2g+1) advance their
            # j-loops together (independent psum accumulators), so the PE has
            # ~6 matmuls in flight per j step to hide each exp's latency.
            # The Wo projection of the previous group is emitted after the
            # current group's attention as additional filler.
            def attn_pair(st, j):
                b, i0, jmax, pv, smp = st
                scp = scps.tile([128, 512], F32, tag="sc")
                nc.tensor.matmul(scp[:], ktb[:, b, j, :],
                                 qtb[:, b, i0:i0 + 2, :, :])
                probs = probsp.tile([128, 512], BF, tag="probs")
                nc.scalar.activation(probs[:], scp[:], AF.Exp)
                if j == i0:
                    nc.vector.tensor_mul(probs[:], probs[:], ma_s[:])
                elif j == jmax:
                    nc.vector.tensor_mul(probs[:], probs[:], mb_s[:])
                nc.tensor.matmul(pv[:], vb[:, b, j, :], probs[:],
                                 start=(j == 0), stop=(j == jmax))
                nc.tensor.matmul(smp[:], o128_s[:], probs[:],
                                 start=(j == 0), stop=(j == jmax))

            def gate_pair(st):
                b, i0, jmax, pv, smp = st
                lsb = awork.tile([128, 512], F32, tag="lsb")
                nc.scalar.activation(lsb[:], smp[:], AF.Ln)
                rsb = awork.tile([128, 512], F32, tag="rsb")
                nc.scalar.activation(rsb[:], lsb[:], AF.Exp, scale=-1.0)
                tmp = awork.tile([128, 512], BF, tag="tmp")
                nc.vector.tensor_mul(tmp[:], pv[:], rsb[:])
                gated = probsp.tile([128, 512], BF, tag="gated")
                nc.vector.tensor_mul(gated[:], tmp[:],
                                     gtb[:, b, i0:i0 + 2, :, :])
                return (b, i0, gated)

            pending = []
            for b in range(B):
                for pA in range(0, NP, 2):
                    pB = pA + 1
                    sts = []
                    for p in (pA, pB):
                        i0 = 2 * p
                        sts.append((b, i0, i0 + 1,
                                    pvps.tile([128, 512], F32, tag="pv",
                                              name="pv"),
                                    sumps.tile([128, 512], F32, tag="sm",
                                               name="sm")))
                    stA, stB = sts
                    done = []
                    for j in range(stB[2] + 1):
                        if j <= stA[2]:
                            attn_pair(stA, j)
                        attn_pair(stB, j)
                        if j == stA[2]:
                            done.append(gate_pair(stA))
                            if pending:
                                wo_proj(*pending.pop(0))
                    done.append(gate_pair(stB))
                    _mark(f'attn b{b}g{pA//2}')
                    for pend in pending:
                        wo_proj(*pend)
                    pending = done
            for pend in pending:
                wo_proj(*pend)
    nc.compile()
    return nc


def prep_inputs(hidden_states, cos, sin, Wq, Wk, Wv, Wo, q_norm_w, k_norm_w,
                S_=S):
    """Host-side sharding + layout prep. Returns in_maps for 8 cores."""
    N = B * S_
    hsT = np.ascontiguousarray(
        hidden_states.reshape(N, HID).T).astype(BF16)

    cos0 = np.asarray(cos[0], np.float32)   # [S_, D] (identical across batch)
    sin0 = np.asarray(sin[0], np.float32)
    qw = np.asarray(q_norm_w, np.float32)
    kw = np.asarray(k_norm_w, np.float32)
    sign = np.where(np.arange(D) < 64, -1.0, 1.0).astype(np.float32)
    shift = (np.arange(D) + 64) % D

    cosq = np.ascontiguousarray(cos0.T * qw[:, None] * SCALE).astype(BF16)
    sinq = np.ascontiguousarray(
        sin0.T * (sign * qw[shift])[:, None] * SCALE).astype(BF16)
    cosk = np.ascontiguousarray(cos0.T * kw[:, None]).astype(BF16)
    sink = np.ascontiguousarray(
        sin0.T * (sign * kw[shift])[:, None]).astype(BF16)

    tri = (np.arange(128)[:, None] <= np.arange(128)[None, :])
    onesq = np.ones((128, 128), np.float32)
    maska = np.concatenate([tri, tri, onesq, onesq], axis=1).astype(BF16)
    maskb = np.concatenate([0 * onesq, 0 * onesq, tri, tri],
                           axis=1).astype(BF16)
    ident = np.eye(128, dtype=BF16)
    ones1 = np.ones((128, 1), BF16)
    onesb = np.ones((1, 128), BF16)
    ones128 = np.ones((128, 128), BF16)

    HC = HID // 128
    in_maps = []
    for d in range(NCORES):
        h0, h1 = G * d, G * d + 1
        q0 = Wq[:, h0 * 2 * D: h0 * 2 * D + D]
        g0 = Wq[:, h0 * 2 * D + D: (h0 + 1) * 2 * D]
        q1 = Wq[:, h1 * 2 * D: h1 * 2 * D + D]
        g1 = Wq[:, h1 * 2 * D + D: (h1 + 1) * 2 * D]
        wq_c = np.concatenate([q0, q1, g0, g1], axis=1)      # [HID, 512]
        wq_a = np.ascontiguousarray(wq_c).astype(BF16).reshape(HC, 128, 512)
        wk_a = np.ascontiguousarray(
            Wk[:, d * D:(d + 1) * D]).astype(BF16).reshape(HC, 128, 128)
        wv_a = np.ascontiguousarray(
            Wv[:, d * D:(d + 1) * D]).astype(BF16).reshape(HC, 128, 128)
        wo_a = np.ascontiguousarray(
            Wo[d * G * D:(d + 1) * G * D, :]).astype(BF16).reshape(G, 128, HID)
        in_maps.append({
            "hsT": hsT, "wq": wq_a, "wk": wk_a, "wv": wv_a, "wo": wo_a,
            "cosq": cosq, "sinq": sinq, "cosk": cosk, "sink": sink,
            "ident": ident, "ones1": ones1, "onesb": onesb,
            "ones128": ones128, "maska": maska, "maskb": maskb,
        })
    return in_maps


_NC_CACHE = {}
_RUNNER_CACHE = {}


def _get_nc(S_=S):
    if S_ not in _NC_CACHE:
        _NC_CACHE[S_] = build_nc(S_)
    return _NC_CACHE[S_]


def _get_runner(S_=S):
    """Build a cached jitted 8-core executable.

    Mirrors concourse.bass2jax.run_bass_via_pjrt's multi-core path, but
    keeps the jitted function (and device-resident output placeholders)
    so repeated calls don't re-trace/re-compile, and so the executable
    can be timed in a steady-state loop.
    """
    if S_ in _RUNNER_CACHE:
        return _RUNNER_CACHE[S_]
    import jax
    from jax.experimental.shard_map import shard_map
    from jax.sharding import Mesh, PartitionSpec
    from concourse import bass2jax, mybir as _mybir
    bass2jax.install_neuronx_cc_hook()

    nc = _get_nc(S_)
    assert nc.dbg_addr is None
    pid_name = (nc.partition_id_tensor.name
                if nc.partition_id_tensor is not None else None)

    in_names, out_names, out_avals = [], [], []
    for alloc in nc.m.functions[0].allocations:
        if not isinstance(alloc, _mybir.MemoryLocationSet):
            continue
        name = alloc.memorylocations[0].name
        if alloc.kind == "ExternalInput":
            if name != pid_name:
                in_names.append(name)
        elif alloc.kind == "ExternalOutput":
            out_names.append(name)
            out_avals.append(jax.core.ShapedArray(
                tuple(alloc.tensor_shape), _mybir.dt.np(alloc.dtype)))
    n_params = len(in_names)
    all_names = in_names + out_names
    if pid_name is not None:
        all_names = all_names + [pid_name]

    def _body(*args):
        operands = list(args)
        if pid_name is not None:
            operands.append(bass2jax.partition_id_tensor())
        outs = bass2jax._bass_exec_p.bind(
            *operands,
            out_avals=tuple(out_avals),
            in_names=tuple(all_names),
            out_names=tuple(out_names),
            lowering_input_output_aliases=(),
            sim_require_finite=True,
            sim_require_nnan=True,
            nc=nc,
        )
        return tuple(outs)

    devices = jax.devices()[:NCORES]
    mesh = Mesh(np.asarray(devices), ("core",))
    nin = n_params + len(out_names)
    sharded = jax.jit(
        shard_map(_body, mesh=mesh,
                  in_specs=(PartitionSpec("core"),) * nin,
                  out_specs=(PartitionSpec("core"),) * len(out_names),
                  check_rep=False),
        keep_unused=True,
    )
    zeros = [np.zeros((NCORES * a.shape[0], *a.shape[1:]), a.dtype)
             for a in out_avals]
    zeros_dev = [jax.device_put(z) for z in zeros]

    def run(in_maps):
        concat_in = [
            np.concatenate([np.asarray(m[nm]) for m in in_maps], axis=0)
            for nm in in_names
        ]
        outs = sharded(*concat_in, *zeros_dev)
        return {nm: np.asarray(outs[i]) for i, nm in enumerate(out_names)}

    def run_prepared(dev_args):
        return sharded(*dev_args, *zeros_dev)

    def prepare(in_maps):
        return [
            jax.device_put(np.concatenate(
                [np.asarray(m[nm]) for m in in_maps], axis=0))
            for nm in in_names
        ]

    r = {"run": run, "prepare": prepare, "run_prepared": run_prepared,
         "out_names": out_names, "out_avals": out_avals}
    _RUNNER_CACHE[S_] = r
    return r


def kernel(hidden_states, cos, sin, Wq, Wk, Wv, Wo, q_norm_w, k_norm_w):
    in_maps = prep_inputs(hidden_states, cos, sin, Wq, Wk, Wv, Wo,
                          q_norm_w, k_norm_w)
    runner = _get_runner()
    outs = runner["run"](in_maps)
    full = outs["out"].reshape(NCORES, B * S, HID)
    acc = full.astype(np.float32).sum(axis=0)
    return acc.reshape(B, S, HID)


# revision 37
# speedup vs baseline: 1.0137x; 1.0062x over previous
"""Qwen-style GQA full attention (B=2, S=2048, HID=2048, H=16, KVH=8, D=128)
on 8 trn2 NeuronCores.

Sharding: tensor-parallel across head groups. Core d owns kv-head d and its
two query heads (2d, 2d+1): Wq/Wk/Wv column shards, Wo row shard. Each core
computes a partial [B*S, HID] output (its 2 heads' contribution through its
Wo row block); the host sums the 8 partials.

Device kernel (per core, all matmuls bf16, fp32 PSUM accumulation):
  phase 1  QKV+gate projection, feature-major ([feat, tok]) via stationary
           W-chunks against moving hsT (host-pretransposed hidden states).
           Per-head RMSNorm done with a ones-vector partition-sum matmul +
           exp(-0.5*ln(ss/128+eps)); RoPE via half-rotated sin/cos tables
           (norm weight + 1/sqrt(D) folded in host-side). Gate sigmoid is
           computed as exp(-ln(1+exp(-g))) so the scalar engine only ever
           needs the natural_log_exp table set.
  phase 2  V transposed to token-major via PE transposes.
  phase 3  causal attention per (batch, q-tile-pair): scoresT = K-chunk
           stationary x moving Q -> exp -> diagonal-block masking (exact
           zeros) -> PV and broadcast row-sum accumulation; out columns are
           rescaled by exp(-ln(sum)) (softmax denominator; no max
           subtraction needed since RMS-normed q,k bound |score|<=sqrt(D)).
  phase 4  sigmoid-gate multiply + Wo row-shard projection -> partial out.
"""

import os
import numpy as np
import ml_dtypes

import concourse.bass as bass
import concourse.tile as tile
from concourse import bacc, mybir
from contextlib import ExitStack

BF16 = ml_dtypes.bfloat16
F32 = mybir.dt.float32
BF = mybir.dt.bfloat16
AF = mybir.ActivationFunctionType

class _Bacc(bacc.Bacc):
    """Bacc that prefers the combined Ln+Exp activation table set, so the
    kernel's Ln/Exp/Copy mix resolves to a single ACT_TABLE_LOAD instead of
    thrashing between exp_and_others and natural_log (~2.7us per switch)."""

    def insert_act_table_loads(self):
        import bass_rust as _bass_rust
        from concourse.hw_specs import get_activation_tables
        has_activation = any(
            isinstance(i, mybir.InstActivation)
            for b in self.main_func.blocks
            for i in b.instructions
        )
        if not has_activation:
            return
        # act_func_set_id is positional: keep list order, but hide every
        # set except the combined one so the pass can only pick it.
        items = [
            (nm, fns if nm == "natural_log_exp_and_others" else set())
            for nm, fns in get_activation_tables(self.m.arch).items()
        ]
        _bass_rust.insert_act_table_loads(self, items)


B, S, HID, H, KVH, D = 2, 2048, 2048, 16, 8, 128
G = H // KVH              # q heads per kv head (= per core)
EPS = 1e-6
SCALE = D ** -0.5
CH = 512                  # token chunk (proj phase)
NCORES = 8


def build_nc(S_=S):
    """Build the single-core SPMD program (identical on all 8 cores)."""
    HC = HID // 128           # hid chunks
    N = B * S_                # total tokens
    SK = S_ // 128            # k-tiles per batch
    NP = S_ // 256            # q-tile pairs per batch
    CPB = S_ // CH            # token chunks per batch
    NT = CH // 128            # 128-tok tiles per chunk

    nc = _Bacc(None)
    nc._phase_marks = []
    _mark = lambda s: nc._phase_marks.append((s, nc.next_id()))

    hsT_d = nc.dram_tensor("hsT", [HID, N], BF, kind="ExternalInput")
    wq_d = nc.dram_tensor("wq", [HC, 128, 512], BF, kind="ExternalInput")
    wk_d = nc.dram_tensor("wk", [HC, 128, 128], BF, kind="ExternalInput")
    wv_d = nc.dram_tensor("wv", [HC, 128, 128], BF, kind="ExternalInput")
    wo_d = nc.dram_tensor("wo", [G, 128, HID], BF, kind="ExternalInput")
    cq_d = nc.dram_tensor("cosq", [128, S_], BF, kind="ExternalInput")
    sq_d = nc.dram_tensor("sinq", [128, S_], BF, kind="ExternalInput")
    ck_d = nc.dram_tensor("cosk", [128, S_], BF, kind="ExternalInput")
    sk_d = nc.dram_tensor("sink", [128, S_], BF, kind="ExternalInput")
    id_d = nc.dram_tensor("ident", [128, 128], BF, kind="ExternalInput")
    o1_d = nc.dram_tensor("ones1", [128, 1], BF, kind="ExternalInput")
    ob_d = nc.dram_tensor("onesb", [1, 128], BF, kind="ExternalInput")
    o128_d = nc.dram_tensor("ones128", [128, 128], BF, kind="ExternalInput")
    ma_d = nc.dram_tensor("maska", [128, 512], BF, kind="ExternalInput")
    mb_d = nc.dram_tensor("maskb", [128, 512], BF, kind="ExternalInput")
    out_d = nc.dram_tensor("out", [N, HID], BF, kind="ExternalOutput")

    with tile.TileContext(nc) as tc, ExitStack() as ctx:
        cpool = ctx.enter_context(tc.tile_pool(name="consts", bufs=1))

        wq_s = cpool.tile([128, HC, 512], BF)
        wk_s = cpool.tile([128, HC, 128], BF)
        wv_s = cpool.tile([128, HC, 128], BF)
        wo_s = cpool.tile([128, G, HID], BF)
        cq_s = cpool.tile([128, S_], BF)
        sq_s = cpool.tile([128, S_], BF)
        ck_s = cpool.tile([128, S_], BF)
        sk_s = cpool.tile([128, S_], BF)
        id_s = cpool.tile([128, 128], BF)
        o1_s = cpool.tile([128, 1], BF)
        ob_s = cpool.tile([1, 128], BF)
        o128_s = cpool.tile([128, 128], BF)
        ma_s = cpool.tile([128, 512], BF)
        mb_s = cpool.tile([128, 512], BF)
        epsb = cpool.tile([128, 1], F32)
        oneb = cpool.tile([128, 1], F32)
        nc.vector.memset(epsb[:], EPS)
        nc.vector.memset(oneb[:], 1.0)

        # per-chunk weight loads so the first projection matmuls unblock
        # as soon as their own W chunk lands (not the whole 2MB tensor)
        for c in range(HC):
            nc.sync.dma_start(wq_s[:, c, :], wq_d[c])
            nc.scalar.dma_start(wk_s[:, c, :], wk_d[c])
            nc.scalar.dma_start(wv_s[:, c, :], wv_d[c])
        nc.sync.dma_start(wo_s[:], wo_d[:].rearrange("c p f -> p c f"))
        for dst, src in ((cq_s, cq_d), (sq_s, sq_d), (ck_s, ck_d), (sk_s, sk_d),
                         (id_s, id_d), (o1_s, o1_d), (ob_s, ob_d),
                         (o128_s, o128_d), (ma_s, ma_d), (mb_s, mb_d)):
            nc.sync.dma_start(dst[:], src[:])

        _mark('consts')
        # persistent activations (feature-major: [D, ...tok])
        qtb = cpool.tile([128, B, SK, G, 128], BF)   # rope'd+normed q
        ktb = cpool.tile([128, B, SK, 128], BF)      # rope'd+normed k
        vtb = cpool.tile([128, N], BF)               # v, feature-major
        vb = cpool.tile([128, B, SK, 128], BF)       # v, token-major
        gtb = cpool.tile([128, B, SK, G, 128], BF)   # sigmoid(gate)

        # ---------------- phase 1: projections ----------------
        hsT_v = hsT_d[:].rearrange("(c p) n -> c p n", p=128)
        with (
            tc.tile_pool(name="hst", bufs=2) as hstp,
            tc.tile_pool(name="projps", bufs=6, space="PSUM") as projps,
            tc.tile_pool(name="ssps", bufs=1, space="PSUM") as ssps,
            tc.tile_pool(name="auxps", bufs=1, space="PSUM") as auxps,
            tc.tile_pool(name="pwork", bufs=3) as pwork,
        ):
            for b in range(B):
                for cc in range(CPB):
                    t0 = b * S_ + cc * CH     # global token start
                    p0 = cc * CH              # position start (within batch)
                    ht = hstp.tile([128, HC, CH], BF, tag="hst")
                    # finer sub-DMAs for the very first chunk so the first
                    # projection matmuls unblock as early as possible
                    step = 2 if (b == 0 and cc == 0) else 4
                    for c4 in range(0, HC, step):
                        nc.gpsimd.dma_start(
                            ht[:, c4:c4 + step, :],
                            hsT_v[c4:c4 + step, :, t0:t0 + CH].rearrange(
                                "c p f -> p c f"))
                    hts = [ht[:, c, :] for c in range(HC)]

                    psq0 = projps.tile([128, CH], F32, tag="pp")
                    psq1 = projps.tile([128, CH], F32, tag="pp")
                    psk = projps.tile([128, CH], F32, tag="pp")
                    psv = projps.tile([128, CH], F32, tag="pp")
                    psg0 = projps.tile([128, CH], F32, tag="pp")
                    psg1 = projps.tile([128, CH], F32, tag="pp")
                    for c in range(HC):
                        st, sp = c == 0, c == HC - 1
                        nc.tensor.matmul(psq0[:], wq_s[:, c, 0:128], hts[c],
                                         start=st, stop=sp)
                        nc.tensor.matmul(psq1[:], wq_s[:, c, 128:256], hts[c],
                                         start=st, stop=sp)
                        nc.tensor.matmul(psk[:], wk_s[:, c, :], hts[c],
                                         start=st, stop=sp)
                        nc.tensor.matmul(psv[:], wv_s[:, c, :], hts[c],
                                         start=st, stop=sp)
                        nc.tensor.matmul(psg0[:], wq_s[:, c, 256:384], hts[c],
                                         start=st, stop=sp)
                        nc.tensor.matmul(psg1[:], wq_s[:, c, 384:512], hts[c],
                                         start=st, stop=sp)

                    ti0 = cc * NT
                    # RMSNorm + RoPE for q heads and k
                    blocks = [
                        (psq0, cq_s, sq_s, qtb[:, b, ti0:ti0 + NT, 0, :]),
                        (psq1, cq_s, sq_s, qtb[:, b, ti0:ti0 + NT, 1, :]),
                        (psk, ck_s, sk_s, ktb[:, b, ti0:ti0 + NT, :]),
                    ]
                    for psx, ctab, stab, dest in blocks:
                        xu = pwork.tile([128, CH], BF, tag="xu")
                        nc.scalar.copy(xu[:], psx[:])
                        xsq = pwork.tile([128, CH], BF, tag="xsq")
                        nc.vector.tensor_mul(xsq[:], xu[:], xu[:])
                        ssp = pwork.tile([1, CH], F32, tag="ssp")
                        nc.gpsimd.tensor_reduce(ssp[:], xsq[:],
                                                mybir.AxisListType.C,
                                                mybir.AluOpType.add)
                        ssl = pwork.tile([1, CH], F32, tag="ssl")
                        nc.scalar.activation(ssl[:], ssp[:], AF.Ln,
                                             bias=epsb[:1], scale=1.0 / D)
                        rsts = pwork.tile([1, CH], BF, tag="rsts")
                        nc.scalar.activation(rsts[:], ssl[:], AF.Exp, scale=-0.5)
                        rstdB = auxps.tile([128, CH], F32, tag="aux")
                        nc.tensor.matmul(rstdB[:], ob_s[:], rsts[:])
                        t1 = pwork.tile([128, CH], BF, tag="t1")
                        nc.vector.tensor_mul(t1[:], xu[:], ctab[:, p0:p0 + CH])
                        xrot = pwork.tile([128, CH], BF, tag="xrot")
                        nc.vector.tensor_copy(xrot[0:64, :], xu[64:128, :])
                        nc.vector.tensor_copy(xrot[64:128, :], xu[0:64, :])
                        t2 = pwork.tile([128, CH], BF, tag="t2")
                        nc.vector.tensor_mul(t2[:], xrot[:],
                                             stab[:, p0:p0 + CH])
                        nc.vector.tensor_add(t1[:], t1[:], t2[:])
                        nc.vector.tensor_mul(dest, t1[:], rstdB[:])

                    # v: stash feature-major (transposed later)
                    nc.scalar.copy(vtb[:, t0:t0 + CH], psv[:])

                    # gates: sigmoid(g) = exp(-ln(1 + exp(-g)))
                    for hh, psg in ((0, psg0), (1, psg1)):
                        e1 = pwork.tile([128, CH], BF, tag="e1")
                        nc.scalar.activation(e1[:], psg[:], AF.Exp, scale=-1.0)
                        l1 = pwork.tile([128, CH], F32, tag="l1")
                        nc.scalar.activation(l1[:], e1[:], AF.Ln, bias=oneb[:])
                        nc.scalar.activation(gtb[:, b, ti0:ti0 + NT, hh, :],
                                             l1[:], AF.Exp, scale=-1.0)
                    _mark(f'proj b{b}c{cc}')

            # ---------------- phase 2: V -> token-major ----------------
            for b in range(B):
                for j4 in range(0, SK, 4):
                    vt_ps = auxps.tile([128, 512], BF, tag="aux",
                                       name="vt_ps")
                    for jj in range(4):
                        j = j4 + jj
                        nc.tensor.transpose(
                            vt_ps[:, jj * 128:(jj + 1) * 128],
                            vtb[:, b * S_ + j * 128: b * S_ + (j + 1) * 128],
                            id_s[:])
                    nc.scalar.copy(vb[:, b, j4:j4 + 4, :], vt_ps[:])

        _mark('vtrans')
        # ---------------- phase 3+4: attention + gating + Wo ----------------
        with (
            tc.tile_pool(name="scps", bufs=2, space="PSUM") as scps,
            tc.tile_pool(name="pvps", bufs=2, space="PSUM") as pvps,
            tc.tile_pool(name="sumps", bufs=2, space="PSUM") as sumps,
            tc.tile_pool(name="wops", bufs=2, space="PSUM") as wops,
            tc.tile_pool(name="probsp", bufs=6) as probsp,
            tc.tile_pool(name="awork", bufs=3) as awork,
        ):
            def wo_proj(b, i0, gated):
                # gating result of pair (b, i0//2) -> Wo row-shard -> DRAM
                for it in range(2):
                    trow = b * S_ + (i0 + it) * 128
                    osb = awork.tile([128, HID], BF, tag="osb")
                    for oc in range(HID // 512):
                        wop = wops.tile([128, 512], F32, tag="wo")
                        nc.tensor.matmul(
                            wop[:], gated[:, it * 256:it * 256 + 128],
                            wo_s[:, 0, oc * 512:(oc + 1) * 512],
                            start=True, stop=False)
                        nc.tensor.matmul(
                            wop[:], gated[:, it * 256 + 128:it * 256 + 256],
                            wo_s[:, 1, oc * 512:(oc + 1) * 512],
                            start=False, stop=True)
                        nc.vector.tensor_copy(
                            osb[:, oc * 512:(oc + 1) * 512], wop[:])
                    nc.gpsimd.dma_start(out_d[trow:trow + 128, :], osb[:])

            # Two-pair interleaved attention: pairs (2g, 2g+1) advance their
            # j-loops together (independent psum accumulators), so the PE has
            # ~6 matmuls in flight per j step to hide each exp's latency.
            # The Wo projection of the previous group is emitted after the
            # current group's attention as additional filler.
            def attn_pair(st, j):
                b, i0, jmax, pv, smp = st
                scp = scps.tile([128, 512], F32, tag="sc")
                nc.tensor.matmul(scp[:], ktb[:, b, j, :],
                                 qtb[:, b, i0:i0 + 2, :, :])
                probs = probsp.tile([128, 512], BF, tag="probs")
                nc.scalar.activation(probs[:], scp[:], AF.Exp)
                if j == i0:
                    nc.vector.tensor_mul(probs[:], probs[:], ma_s[:])
                elif j == jmax:
                    nc.vector.tensor_mul(probs[:], probs[:], mb_s[:])
                nc.tensor.matmul(pv[:], vb[:, b, j, :], probs[:],
                                 start=(j == 0), stop=(j == jmax))
                nc.tensor.matmul(smp[:], o128_s[:], probs[:],
                                 start=(j == 0), stop=(j == jmax))

            def gate_pair(st):
                b, i0, jmax, pv, smp = st
                lsb = awork.tile([128, 512], F32, tag="lsb")
                nc.scalar.activation(lsb[:], smp[:], AF.Ln)
                rsb = awork.tile([128, 512], F32, tag="rsb")
                nc.scalar.activation(rsb[:], lsb[:], AF.Exp, scale=-1.0)
                tmp = awork.tile([128, 512], BF, tag="tmp")
                nc.vector.tensor_mul(tmp[:], pv[:], rsb[:])
                gated = probsp.tile([128, 512], BF, tag="gated")
                nc.vector.tensor_mul(gated[:], tmp[:],
                                     gtb[:, b, i0:i0 + 2, :, :])
                return (b, i0, gated)

            pending = []
            for b in range(B):
                for pA in range(0, NP, 2):
                    pB = pA + 1
                    sts = []
                    for p in (pA, pB):
                        i0 = 2 * p
                        sts.append((b, i0, i0 + 1,
                                    pvps.tile([128, 512], F32, tag="pv",
                                              name="pv"),
                                    sumps.tile([128, 512], F32, tag="sm",
                                               name="sm")))
                    stA, stB = sts
                    done = []
                    for j in range(stB[2] + 1):
                        if j <= stA[2]:
                            attn_pair(stA, j)
                        attn_pair(stB, j)
                        if j == stA[2]:
                            done.append(gate_pair(stA))
                            if pending:
                                wo_proj(*pending.pop(0))
                    done.append(gate_pair(stB))
                    _mark(f'attn b{b}g{pA//2}')
                    for pend in pending:
                        wo_proj(*pend)
                    pending = done
            for pend in pending:
                wo_proj(*pend)
    nc.compile()
    return nc


def prep_inputs(hidden_states, cos, sin, Wq, Wk, Wv, Wo, q_norm_w, k_norm_w,
                S_=S):
    """Host-side sharding + layout prep. Returns in_maps for 8 cores."""
    N = B * S_
    hsT = np.ascontiguousarray(
        hidden_states.reshape(N, HID).T).astype(BF16)

    cos0 = np.asarray(cos[0], np.float32)   # [S_, D] (identical across batch)
    sin0 = np.asarray(sin[0], np.float32)
    qw = np.asarray(q_norm_w, np.float32)
    kw = np.asarray(k_norm_w, np.float32)
    sign = np.where(np.arange(D) < 64, -1.0, 1.0).astype(np.float32)
    shift = (np.arange(D) + 64) % D

    cosq = np.ascontiguousarray(cos0.T * qw[:, None] * SCALE).astype(BF16)
    sinq = np.ascontiguousarray(
        sin0.T * (sign * qw[shift])[:, None] * SCALE).astype(BF16)
    cosk = np.ascontiguousarray(cos0.T * kw[:, None]).astype(BF16)
    sink = np.ascontiguousarray(
        sin0.T * (sign * kw[shift])[:, None]).astype(BF16)

    tri = (np.arange(128)[:, None] <= np.arange(128)[None, :])
    onesq = np.ones((128, 128), np.float32)
    maska = np.concatenate([tri, tri, onesq, onesq], axis=1).astype(BF16)
    maskb = np.concatenate([0 * onesq, 0 * onesq, tri, tri],
                           axis=1).astype(BF16)
    ident = np.eye(128, dtype=BF16)
    ones1 = np.ones((128, 1), BF16)
    onesb = np.ones((1, 128), BF16)
    ones128 = np.ones((128, 128), BF16)

    HC = HID // 128
    in_maps = []
    for d in range(NCORES):
        h0, h1 = G * d, G * d + 1
        q0 = Wq[:, h0 * 2 * D: h0 * 2 * D + D]
        g0 = Wq[:, h0 * 2 * D + D: (h0 + 1) * 2 * D]
        q1 = Wq[:, h1 * 2 * D: h1 * 2 * D + D]
        g1 = Wq[:, h1 * 2 * D + D: (h1 + 1) * 2 * D]
        wq_c = np.concatenate([q0, q1, g0, g1], axis=1)      # [HID, 512]
        wq_a = np.ascontiguousarray(wq_c).astype(BF16).reshape(HC, 128, 512)
        wk_a = np.ascontiguousarray(
            Wk[:, d * D:(d + 1) * D]).astype(BF16).reshape(HC, 128, 128)
        wv_a = np.ascontiguousarray(
            Wv[:, d * D:(d + 1) * D]).astype(BF16).reshape(HC, 128, 128)
        wo_a = np.ascontiguousarray(
            Wo[d * G * D:(d + 1) * G * D, :]).astype(BF16).reshape(G, 128, HID)
        in_maps.append({
            "hsT": hsT, "wq": wq_a, "wk": wk_a, "wv": wv_a, "wo": wo_a,
            "cosq": cosq, "sinq": sinq, "cosk": cosk, "sink": sink,
            "ident": ident, "ones1": ones1, "onesb": onesb,
            "ones128": ones128, "maska": maska, "maskb": maskb,
        })
    return in_maps


_NC_CACHE = {}
_RUNNER_CACHE = {}


def _get_nc(S_=S):
    if S_ not in _NC_CACHE:
        _NC_CACHE[S_] = build_nc(S_)
    return _NC_CACHE[S_]


def _get_runner(S_=S):
    """Build a cached jitted 8-core executable.

    Mirrors concourse.bass2jax.run_bass_via_pjrt's multi-core path, but
    keeps the jitted function (and device-resident output placeholders)
    so repeated calls don't re-trace/re-compile, and so the executable
    can be timed in a steady-state loop.
    """
    if S_ in _RUNNER_CACHE:
        return _RUNNER_CACHE[S_]
    import jax
    from jax.experimental.shard_map import shard_map
    from jax.sharding import Mesh, PartitionSpec
    from concourse import bass2jax, mybir as _mybir
    bass2jax.install_neuronx_cc_hook()

    nc = _get_nc(S_)
    assert nc.dbg_addr is None
    pid_name = (nc.partition_id_tensor.name
                if nc.partition_id_tensor is not None else None)

    in_names, out_names, out_avals = [], [], []
    for alloc in nc.m.functions[0].allocations:
        if not isinstance(alloc, _mybir.MemoryLocationSet):
            continue
        name = alloc.memorylocations[0].name
        if alloc.kind == "ExternalInput":
            if name != pid_name:
                in_names.append(name)
        elif alloc.kind == "ExternalOutput":
            out_names.append(name)
            out_avals.append(jax.core.ShapedArray(
                tuple(alloc.tensor_shape), _mybir.dt.np(alloc.dtype)))
    n_params = len(in_names)
    all_names = in_names + out_names
    if pid_name is not None:
        all_names = all_names + [pid_name]

    def _body(*args):
        operands = list(args)
        if pid_name is not None:
            operands.append(bass2jax.partition_id_tensor())
        outs = bass2jax._bass_exec_p.bind(
            *operands,
            out_avals=tuple(out_avals),
            in_names=tuple(all_names),
            out_names=tuple(out_names),
            lowering_input_output_aliases=(),
            sim_require_finite=True,
            sim_require_nnan=True,
            nc=nc,
        )
        return tuple(outs)

    devices = jax.devices()[:NCORES]
    mesh = Mesh(np.asarray(devices), ("core",))
    nin = n_params + len(out_names)
    sharded = jax.jit(
        shard_map(_body, mesh=mesh,
                  in_specs=(PartitionSpec("core"),) * nin,
                  out_specs=(PartitionSpec("core"),) * len(out_names),
                  check_rep=False),
        keep_unused=True,
    )
    zeros = [np.zeros((NCORES * a.shape[0], *a.shape[1:]), a.dtype)
             for a in out_avals]
    zeros_dev = [jax.device_put(z) for z in zeros]

    def run(in_maps):
        concat_in = [
            np.concatenate([np.asarray(m[nm]) for m in in_maps], axis=0)
            for nm in in_names
        ]
        outs = sharded(*concat_in, *zeros_dev)
        return {nm: np.asarray(outs[i]) for i, nm in enumerate(out_names)}

    def run_prepared(dev_args):
        return sharded(*dev_args, *zeros_dev)

    def prepare(in_maps):
        return [
            jax.device_put(np.concatenate(
                [np.asarray(m[nm]) for m in in_maps], axis=0))
            for nm in in_names
        ]

    r = {"run": run, "prepare": prepare, "run_prepared": run_prepared,
         "out_names": out_names, "out_avals": out_avals}
    _RUNNER_CACHE[S_] = r
    return r


def kernel(hidden_states, cos, sin, Wq, Wk, Wv, Wo, q_norm_w, k_norm_w):
    in_maps = prep_inputs(hidden_states, cos, sin, Wq, Wk, Wv, Wo,
                          q_norm_w, k_norm_w)
    runner = _get_runner()
    outs = runner["run"](in_maps)
    full = outs["out"].reshape(NCORES, B * S, HID)
    acc = full.astype(np.float32).sum(axis=0)
    return acc.reshape(B, S, HID)
